# revision 20
# baseline (speedup 1.0000x reference)
"""Trainium2 Bass kernel for nn_Discriminator (GNN message passing).

Math (reference):
    h   = relu(embedding @ W_emb + b_emb)          # [N, HID]
    w_sym = 0.5*(W_edge[:HID,0] + W_edge[HID:,0])  # [HID]
    raw = (h[e0] + h[e1]) @ w_sym + b_edge         # [E]
    out = sigmoid(logit(eps) + raw),  eps = (2B-1)*u + (1-B)

Algebraic reduction: raw[e] = s[e0] + s[e1] + b_edge with per-node scalar
s = h @ w_sym, so the edge stage only needs per-node scalars from a 53k
table.

Distribution (8 NeuronCores):
  - node GEMM sharded over N (6250 nodes/core, padded 6656)
  - AllGather of s -> full table on every core
  - edges assigned to cores/slots by the host (output unpermuted on host)

Edge stage (the fast part, replacing the v1 ap_gather approach):
  The s-table sits in SBUF as [128, 3328] fp16: partition 16g+q holds
  table chunk q (replicated over the 8 gpsimd groups g).  Each edge is
  assigned a slot (group g, column J); its two endpoint values are routed
  into V[16g+c, J] (c = endpoint's chunk) by two local_scatter hops
  (table -> X -> V), both with host-precomputed index maps.  A single
  ones-vector matmul per 450-column window sums the 16 chunk rows of each
  group into PSUM, yielding raw[g, J].  local_scatter runs vectorized in
  GPSIMD local RAM (~0.3ns/elem streamed) instead of ap_gather's ~28ns
  per random SBUF read, which was the v1 bottleneck (~700us).
  Edges whose endpoints collide in the same table chunk use even/odd
  column pairs (summed by a strided DVE pass); assignment stragglers and
  self-loops go through a tiny ap_gather overflow lane (64 slots/group).
"""

import os
import sys
import types
import contextlib
import ctypes

sys.path.insert(0, "/opt/trn_rl_repo")

import numpy as np

import concourse.bass as bass
import concourse.mybir as mybir
import concourse.tile as tile
import concourse.bacc as bacc
from concourse.bass_utils import run_bass_kernel_spmd

# ---------------------------------------------------------------- constants
N, IN_DIM, HID, E = 50000, 512, 256, 800000
NCORES = 8
BIAS = 0.0001

NLOC = N // NCORES          # 6250 real nodes per core
SLABS = 13                  # s staging rows of 512 (13*512 = 6656)
NLOC_PAD = SLABS * 512      # 6656 padded local nodes
RANK_PAD = SLABS * 512
NPAD = RANK_PAD * NCORES    # 53248 = 16*3328
CHUNK = NPAD // 16          # 3328 table entries per chunk

J = 14400                   # slot columns per group
PEW = 450                   # PE window
NPE = J // PEW              # 32 PE windows
VW = 1800                   # V piece width (scatter2 dst)
NW = J // VW                # 8 V pieces
XS = 254                    # X section per window
XW = NW * XS                # 2032  (2032*32 = 65024 < 65536)
S_CAP = 12150               # singles region J in [0, 12150)
P_LO, P_HI = 12150, 13950   # pairs region (even/odd J pairs)
OVF_LO = 13950              # overflow region start
OVF_CAP = 64                # overflow slots per group (gather 64 idx x2)
FOLD = J * 8 // 128         # 900 folded columns per partition

f32 = mybir.dt.float32
f16 = mybir.dt.float16
f32r = mybir.dt.float32r
i16 = mybir.dt.int16
u8 = mybir.dt.uint8


def _install_ntff_hook():
    """Provide antenv.axon_hooks (absent in this image) so trace=True works."""
    if "antenv.axon_hooks" in sys.modules:
        return
    try:
        lib = ctypes.CDLL("/opt/axon/libaxon_pjrt.so")
    except OSError:
        return
    if not hasattr(lib, "axon_start_nrt_profile"):
        return
    lib.axon_start_nrt_profile.argtypes = [ctypes.POINTER(ctypes.c_int64), ctypes.c_size_t]
    lib.axon_start_nrt_profile.restype = ctypes.c_int64
    lib.axon_stop_nrt_profile.argtypes = [ctypes.c_char_p]
    lib.axon_stop_nrt_profile.restype = ctypes.c_int64

    @contextlib.contextmanager
    def _hook(output_dir, device_ids):
        import jax
        jax.devices()
        if device_ids:
            ids = (ctypes.c_int64 * len(device_ids))(*device_ids)
            rc = lib.axon_start_nrt_profile(ids, len(device_ids))
        else:
            rc = lib.axon_start_nrt_profile(None, 0)
        if rc != 0:
            raise RuntimeError(f"axon_start_nrt_profile rc={rc}")
        try:
            yield
        finally:
            n = lib.axon_stop_nrt_profile(str(output_dir).encode())
            print(f"profile: {n} file(s) written to {output_dir}", file=sys.stderr)

    mod = types.ModuleType("antenv.axon_hooks")
    mod.get_axon_ntff_profile_hook = lambda: _hook
    mod.set_axon_ntff_profile_hook = lambda h: None
    sys.modules["antenv.axon_hooks"] = mod


_install_ntff_hook()

_PROGRAM_CACHE = {}


def _build_program():
    nc = bacc.Bacc(None)

    embT = nc.dram_tensor("embT", [4 * SLABS * 128, 512], f32r, kind="ExternalInput")
    Wt = nc.dram_tensor("Wt", [IN_DIM, HID], f32r, kind="ExternalInput")
    bias2 = nc.dram_tensor("bias2", [128, 2], f32, kind="ExternalInput")
    wsym2 = nc.dram_tensor("wsym2", [128, 2], f32r, kind="ExternalInput")
    bedge = nc.dram_tensor("bedge", [128, 3], f32, kind="ExternalInput")
    iota16 = nc.dram_tensor("iota16", [128, 1], f32, kind="ExternalInput")
    e8h = nc.dram_tensor("e8h", [128, 8], f16, kind="ExternalInput")
    e8f = nc.dram_tensor("e8f", [128, 8], f32r, kind="ExternalInput")
    idx1 = nc.dram_tensor("idx1", [128, CHUNK], i16, kind="ExternalInput")
    idx2 = nc.dram_tensor("idx2", [128, XW], i16, kind="ExternalInput")
    ogidxa = nc.dram_tensor("ogidxa", [128, OVF_CAP // 16], i16, kind="ExternalInput")
    ogidxb = nc.dram_tensor("ogidxb", [128, OVF_CAP // 16], i16, kind="ExternalInput")
    ocfa = nc.dram_tensor("ocfa", [128, PEW], u8, kind="ExternalInput")
    ocfb = nc.dram_tensor("ocfb", [128, PEW], u8, kind="ExternalInput")
    uu = nc.dram_tensor("uu", [128, FOLD], f32, kind="ExternalInput")
    out = nc.dram_tensor("out", [128, FOLD], f32, kind="ExternalOutput")

    with tile.TileContext(nc) as tc:
        with (
            tc.tile_pool(name="const", bufs=1) as constp,
            tc.tile_pool(name="w", bufs=1) as wp,
            tc.tile_pool(name="emb", bufs=3) as embp,
            tc.tile_pool(name="h", bufs=2) as hp,
            tc.tile_pool(name="s", bufs=3) as sp,
            tc.tile_pool(name="tab", bufs=1) as tabp,
            tc.tile_pool(name="x", bufs=1) as xp,
            tc.tile_pool(name="v", bufs=3) as vp,
            tc.tile_pool(name="fin", bufs=1) as finp,
            tc.tile_pool(name="psA", bufs=2, space="PSUM") as psA,
            tc.tile_pool(name="psS", bufs=2, space="PSUM") as psS,
            tc.tile_pool(name="psR", bufs=4, space="PSUM") as psR,
            tc.tile_pool(name="dram", bufs=1, space="DRAM") as dramp,
        ):
            # ---------------- constants into SBUF
            t_bias2 = constp.tile([128, 2], f32)
            nc.sync.dma_start(t_bias2[:], bias2[:])
            t_wsym2 = constp.tile([128, 2], f32r)
            nc.sync.dma_start(t_wsym2[:], wsym2[:])
            t_bedge = constp.tile([128, 3], f32)
            nc.sync.dma_start(t_bedge[:], bedge[:])
            t_iota16 = constp.tile([128, 1], f32)
            nc.sync.dma_start(t_iota16[:], iota16[:])
            t_e8h = constp.tile([128, 8], f16)
            nc.sync.dma_start(t_e8h[:], e8h[:])
            t_e8f = constp.tile([128, 8], f32r)
            nc.sync.dma_start(t_e8f[:], e8f[:])
            t_W = wp.tile([128, 4 * HID], f32r)
            for k in range(4):
                nc.sync.dma_start(t_W[:, k * HID:(k + 1) * HID], Wt[128 * k:128 * (k + 1), :])
            # index maps on the Activation DMA queue to spread dispatch
            t_idx1 = constp.tile([128, CHUNK], i16)
            nc.scalar.dma_start(t_idx1[:], idx1[:])
            t_idx2 = constp.tile([128, XW], i16)
            nc.scalar.dma_start(t_idx2[:], idx2[:])
            t_ogidxa = constp.tile([128, OVF_CAP // 16], i16)
            nc.scalar.dma_start(t_ogidxa[:], ogidxa[:])
            t_ogidxb = constp.tile([128, OVF_CAP // 16], i16)
            nc.scalar.dma_start(t_ogidxb[:], ogidxb[:])
            t_ocfa = constp.tile([128, PEW], u8)
            nc.scalar.dma_start(t_ocfa[:], ocfa[:])
            t_ocfb = constp.tile([128, PEW], u8)
            nc.scalar.dma_start(t_ocfb[:], ocfb[:])
            t_u = finp.tile([128, FOLD], f32)
            nc.scalar.dma_start(t_u[:], uu[:])

            # ---------------- stage A: s = relu(emb @ W + b) @ w_sym
            d_sin = dramp.tile([SLABS, 512], f16)
            for si in range(SLABS):
                t_embs = embp.tile([128, 4 * 512], f32r, tag="embs")
                for k in range(4):
                    blk = (si * 4 + k) * 128
                    nc.sync.dma_start(
                        t_embs[:, k * 512:(k + 1) * 512],
                        embT[blk:blk + 128, :],
                    )
                ps_s = psS.tile([1, 512], f32, tag="ps_s")
                for H in range(2):
                    ps_h = psA.tile([128, 512], f32, tag="ps_h")
                    for k in range(4):
                        nc.tensor.matmul(
                            ps_h[:],
                            lhsT=t_W[:, k * HID + 128 * H:k * HID + 128 * (H + 1)],
                            rhs=t_embs[:, k * 512:(k + 1) * 512],
                            start=(k == 0),
                            stop=(k == 3),
                        )
                    t_h = hp.tile([128, 512], f32r, tag="h")
                    nc.scalar.activation(
                        t_h[:], ps_h[:],
                        mybir.ActivationFunctionType.Relu,
                        bias=t_bias2[:, H:H + 1],
                    )
                    nc.tensor.matmul(
                        ps_s[:1, :],
                        lhsT=t_wsym2[:, H:H + 1],
                        rhs=t_h[:],
                        start=(H == 0),
                        stop=(H == 1),
                    )
                t_sst = sp.tile([1, 512], f16, tag="sst")
                nc.vector.tensor_copy(t_sst[:1, :], ps_s[:1, :])
                nc.sync.dma_start(d_sin[si:si + 1, :], t_sst[:1, :])

            # ---------------- stage B: AllGather s (fp16) -> full table
            d_sout = dramp.tile([16, CHUNK], f16)
            nc.gpsimd.collective_compute(
                "AllGather",
                mybir.AluOpType.bypass,
                ins=[d_sin[:].opt()],
                outs=[d_sout[:].opt()],
                replica_groups=[list(range(NCORES))],
            )
            t_tabh = tabp.tile([128, CHUNK], f16)
            for g in range(8):
                nc.sync.dma_start(t_tabh[16 * g:16 * (g + 1), :], d_sout[:, :])
            # f32 table (overflow gather source) cast up from fp16
            t_tabf = tabp.tile([128, CHUNK], f32)
            nc.vector.tensor_copy(t_tabf[:], t_tabh[:])

            # ---------------- overflow lane: tiny ap_gather (async on gpsimd)
            t_oga = tabp.tile([128, PEW], f32)
            t_ogb = tabp.tile([128, PEW], f32)
            nc.vector.memset(t_oga[:, OVF_CAP:], 0.0)
            nc.vector.memset(t_ogb[:, OVF_CAP:], 0.0)
            tabf3 = t_tabf[:].rearrange("p (n d) -> p n d", d=1)
            nc.gpsimd.ap_gather(
                t_oga[:, :OVF_CAP].rearrange("p (n d) -> p n d", d=1),
                tabf3, t_ogidxa[:],
                channels=128, num_elems=CHUNK, d=1, num_idxs=OVF_CAP)
            nc.gpsimd.ap_gather(
                t_ogb[:, :OVF_CAP].rearrange("p (n d) -> p n d", d=1),
                tabf3, t_ogidxb[:],
                channels=128, num_elems=CHUNK, d=1, num_idxs=OVF_CAP)
            # masks: keep only the partition whose chunk matches
            t_cfa = tabp.tile([128, PEW], f32)
            nc.vector.tensor_copy(t_cfa[:], t_ocfa[:])
            t_cfb = tabp.tile([128, PEW], f32)
            nc.vector.tensor_copy(t_cfb[:], t_ocfb[:])
            t_mA = tabp.tile([128, PEW], f32r)
            nc.vector.scalar_tensor_tensor(
                t_mA[:], in0=t_cfa[:], scalar=t_iota16[:, 0:1], in1=t_oga[:],
                op0=mybir.AluOpType.is_equal, op1=mybir.AluOpType.mult)
            t_mB = tabp.tile([128, PEW], f32r)
            nc.vector.scalar_tensor_tensor(
                t_mB[:], in0=t_cfb[:], scalar=t_iota16[:, 0:1], in1=t_ogb[:],
                op0=mybir.AluOpType.is_equal, op1=mybir.AluOpType.mult)

            # ---------------- edge main: scatter1 (table -> X)
            t_x = xp.tile([128, XW], f16)
            nc.gpsimd.local_scatter(
                t_x[:], t_tabh[:], t_idx1[:],
                channels=128, num_elems=XW, num_idxs=CHUNK)

            # ---------------- scatter2 pieces + PE reduce
            # window win [8, 450] lands in folded t_raw at partitions
            # {16g + win//2}, cols [450*(win%2), +450)
            t_raw = finp.tile([128, FOLD], f32)
            raw3 = t_raw[:].rearrange("(g c) f -> g c f", g=8)

            def fold_dst(win):
                return raw3[:, win // 2, PEW * (win % 2):PEW * (win % 2 + 1)]

            for w in range(NW):
                t_v = vp.tile([128, VW], f16, tag="v")
                nc.gpsimd.local_scatter(
                    t_v[:], t_x[:, XS * w:XS * (w + 1)], t_idx2[:, XS * w:XS * (w + 1)],
                    channels=128, num_elems=VW, num_idxs=XS)
                for k in range(4):
                    win = 4 * w + k
                    ps_r = psR.tile([8, PEW], f32, tag="ps_r")
                    last_win = (win == NPE - 1)
                    nc.tensor.matmul(
                        ps_r[:], lhsT=t_e8h[:],
                        rhs=t_v[:, PEW * k:PEW * (k + 1)],
                        start=True, stop=not last_win)
                    if last_win:  # overflow strips accumulate into win 31
                        nc.tensor.matmul(ps_r[:], lhsT=t_e8f[:], rhs=t_mA[:],
                                         start=False, stop=False)
                        nc.tensor.matmul(ps_r[:], lhsT=t_e8f[:], rhs=t_mB[:],
                                         start=False, stop=True)
                    t_r = sp.tile([8, PEW], f32, tag="raw")
                    nc.vector.tensor_copy(t_r[:], ps_r[:])
                    if win in (27, 29):
                        # pair window A: hold; summed with partner window B
                        pend_pair = t_r
                        continue
                    if win in (28, 30):
                        # pairs: raw[J] = raw_A[J] + raw_B[J+450]
                        t_f = sp.tile([8, PEW], f32, tag="fix")
                        nc.vector.tensor_add(t_f[:], pend_pair[:], t_r[:])
                        nc.sync.dma_start(fold_dst(win - 1), t_f[:])
                    nc.sync.dma_start(fold_dst(win), t_r[:])

            # ---------------- gate: logit(eps) + raw, sigmoid
            # split into halves: cols [0,450) depend only on even windows,
            # so that half overlaps the tail of the scatter/PE chain
            a = 1.0 - 2.0 * BIAS
            HF = FOLD // 2
            t_l1 = finp.tile([128, FOLD], f32)
            nc.scalar.activation(t_l1[:], t_u[:], mybir.ActivationFunctionType.Ln,
                                 bias=t_bedge[:, 1:2], scale=-a)
            t_l2 = finp.tile([128, FOLD], f32)
            nc.scalar.activation(t_l2[:], t_u[:], mybir.ActivationFunctionType.Ln,
                                 bias=t_bedge[:, 2:3], scale=a)
            t_gate = finp.tile([128, FOLD], f32)
            nc.vector.tensor_sub(t_gate[:], t_l1[:], t_l2[:])
            t_gate2 = finp.tile([128, FOLD], f32)
            t_out = finp.tile([128, FOLD], f32)
            for hh in range(2):
                sl = slice(HF * hh, HF * (hh + 1))
                nc.vector.tensor_add(t_gate2[:, sl], t_gate[:, sl], t_raw[:, sl])
                nc.scalar.activation(t_out[:, sl], t_gate2[:, sl],
                                     mybir.ActivationFunctionType.Sigmoid,
                                     bias=t_bedge[:, 0:1])
                nc.sync.dma_start(out[:, sl], t_out[:, sl])

    nc.finalize()
    return nc


# ================================================================ host prep
def _assign_edges(e0, e1, rng):
    """Assign each edge to (bin=core*8+group, kind, J-slot, window).

    Returns dict of per-edge arrays: bin, kind (0=single,1=pair,2=ovf),
    w (V piece), J (slot column; for pairs the even column).
    Constraints honored:
      - per (bin, node): at most one reference (scatter1 is one cell per
        (row, node));
      - per (bin, window, kind): slot-region capacities;
      - per (bin, class, window): at most XS refs (X section capacity);
      - per bin: at most OVF_CAP overflow edges.
    """
    Etot = e0.shape[0]
    ip0 = (RANK_PAD * (e0 // NLOC) + (e0 % NLOC)).astype(np.int64)
    ip1 = (RANK_PAD * (e1 // NLOC) + (e1 % NLOC)).astype(np.int64)
    c0 = (ip0 // CHUNK).astype(np.int32)
    c1 = (ip1 // CHUNK).astype(np.int32)
    selfloop = e0 == e1
    pair = (c0 == c1) & ~selfloop
    kind = np.where(pair, 1, 0).astype(np.int8)
    kind[selfloop] = 2

    NBINS = 64
    bin_of = np.full(Etot, -1, np.int32)
    used = np.zeros(NBINS * NPAD, bool)
    sing_cnt = np.zeros(NBINS, np.int64)
    pair_cnt = np.zeros(NBINS, np.int64)
    S_BIN_CAP, P_BIN_CAP = S_CAP, (P_HI - P_LO) // 2

    un = np.where(~selfloop)[0]
    for _ in range(60):
        if un.size == 0:
            break
        prop = rng.integers(0, NBINS, un.size).astype(np.int64)
        k0 = prop * NPAD + ip0[un]
        k1 = prop * NPAD + ip1[un]
        ok = (~used[k0]) & (~used[k1])
        cand = np.where(ok)[0]
        if cand.size == 0:
            continue
        ck0, ck1 = k0[cand], k1[cand]
        L = cand.size
        flat = np.concatenate([ck0, ck1])
        srt = np.argsort(flat, kind="stable")
        fs = flat[srt]
        firstpos = np.ones(2 * L, bool)
        firstpos[1:] = fs[1:] != fs[:-1]
        first = np.zeros(2 * L, bool)
        first[srt] = firstpos
        win = first[:L] & first[L:]
        wc = cand[win]
        if wc.size == 0:
            continue
        # capacity per (bin, kind)
        wbin = prop[wc]
        wkind = kind[un[wc]]
        order = np.lexsort((np.arange(wc.size), wkind, wbin))
        sb, sk = wbin[order], wkind[order]
        grp = sb * 2 + sk
        newgrp = np.ones(sb.size, bool)
        newgrp[1:] = grp[1:] != grp[:-1]
        gstart = np.maximum.accumulate(np.where(newgrp, np.arange(sb.size), 0))
        cc = np.arange(sb.size) - gstart
        already = np.where(sk == 0, sing_cnt[sb], pair_cnt[sb])
        cap = np.where(sk == 0, S_BIN_CAP, P_BIN_CAP)
        acc_sorted = cc + already < cap
        acc = np.zeros(wc.size, bool)
        acc[order] = acc_sorted
        fin = wc[acc]
        if fin.size == 0:
            continue
        eidx = un[fin]
        b = prop[fin]
        bin_of[eidx] = b
        used[b * NPAD + ip0[eidx]] = True
        used[b * NPAD + ip1[eidx]] = True
        np.add.at(sing_cnt, b[kind[eidx] == 0], 1)
        np.add.at(pair_cnt, b[kind[eidx] == 1], 1)
        un = un[~np.isin(un, eidx, assume_unique=True)]

    # leftovers + selfloops -> overflow
    ovf = np.concatenate([un, np.where(selfloop)[0]])
    kind[ovf] = 2
    if ovf.size > 0:
        prop = rng.permutation(ovf.size) % NBINS
        order = np.argsort(prop, kind="stable")
        sb = prop[order]
        newb = np.ones(sb.size, bool)
        newb[1:] = sb[1:] != sb[:-1]
        gstart = np.maximum.accumulate(np.where(newb, np.arange(sb.size), 0))
        cc = np.arange(sb.size) - gstart
        if (cc >= OVF_CAP).any():
            raise RuntimeError(f"overflow capacity exceeded: {ovf.size} stragglers")
        bin_of[ovf[order]] = sb
        ovf_slot = np.zeros(Etot, np.int32)
        ovf_slot[ovf[order]] = cc.astype(np.int32)
    else:
        ovf_slot = np.zeros(Etot, np.int32)

    # ---------------- window assignment (for singles and pairs)
    # pairs live in PE windows 27..30 as (J, J+450) partners:
    #   pair-group 0: J_A in [12150,12600) [piece 6], J_B=J_A+450 [piece 7]
    #   pair-group 1: J_A in [13050,13500) [piece 7], J_B=J_A+450 [piece 7]
    w_of = np.full(Etot, -1, np.int8)
    # slot capacities per window (singles); pairs capacity per pair-group
    s_wcap = np.array([1800] * 6 + [1350, 0], np.int64)
    p_wcap = np.array([450, 450] + [0] * (NW - 2), np.int64)
    xload = np.zeros(NBINS * 16 * NW, np.int64)   # refs per (bin, class, window)
    swcnt = np.zeros(NBINS * NW, np.int64)
    pwcnt = np.zeros(NBINS * NW, np.int64)
    def _cap_accept(keys, weights, counts, caps):
        """Accept a prefix per key group such that counts[key]+cumw <= caps."""
        order = np.argsort(keys, kind="stable")
        sk = keys[order]
        csw = np.cumsum(weights[order])
        newg = np.ones(sk.size, bool)
        newg[1:] = sk[1:] != sk[:-1]
        gidx = np.where(newg, np.arange(sk.size), 0)
        gstart = np.maximum.accumulate(gidx)
        base = np.where(gstart > 0, csw[np.maximum(gstart - 1, 0)], 0.0)
        base[gstart == 0] = 0.0
        cumw = csw - base  # inclusive cumulative weight within group
        ok_sorted = counts[sk] + cumw <= caps[sk]
        ok = np.zeros(keys.size, bool)
        ok[order] = ok_sorted
        return ok

    todo = np.where(kind != 2)[0]
    for _ in range(60):
        if todo.size == 0:
            break
        k = kind[todo]
        pw = np.where(k == 0, 1.0, 0.0)[:, None] * s_wcap[None, :] + \
             np.where(k == 1, 1.0, 0.0)[:, None] * p_wcap[None, :]
        r = rng.random(todo.size)
        cdf = np.cumsum(pw, axis=1)
        cdf /= cdf[:, -1:]
        prop = (r[:, None] < cdf).argmax(axis=1).astype(np.int64)
        b = bin_of[todo].astype(np.int64)
        bw = b * NW + prop
        # slot capacity per (bin, w, kind)
        wcnt_all = np.concatenate([swcnt, pwcnt])  # (bin*NW+w) + 512*kind
        keys_s = bw + (NBINS * NW) * k.astype(np.int64)
        cap_lookup = np.empty(2 * NBINS * NW, np.int64)
        cap_lookup[:NBINS * NW] = np.tile(s_wcap, NBINS)
        cap_lookup[NBINS * NW:] = np.tile(p_wcap, NBINS)
        ok1 = _cap_accept(keys_s, np.ones(todo.size), wcnt_all, cap_lookup)
        # X-load: joint check on both ref cells (phantom consumption on
        # partial accept is conservative and simply retries next round).
        # singles: (c0, w) and (c1, w); pair group 0: (c0, 6) and (c0, 7);
        # pair group 1: (c0, 7) weight 2.
        rowb0 = b * 16 + c0[todo]
        rowb1 = b * 16 + c1[todo]
        cellA = np.where(k == 0, rowb0 * NW + prop,
                         np.where(prop == 0, rowb0 * NW + 6, rowb0 * NW + 7))
        wgtA = np.where((k == 1) & (prop == 1), 2.0, 1.0)
        cellB = np.where(k == 0, rowb1 * NW + prop, rowb0 * NW + 7)
        wgtB = np.where(k == 0, 1.0, np.where(prop == 0, 1.0, 0.0))
        xcap = np.full(NBINS * 16 * NW, XS, np.int64)
        sub = np.where(ok1)[0]
        L = sub.size
        xkeys = np.concatenate([cellA[sub], cellB[sub]])
        xwgt = np.concatenate([wgtA[sub], wgtB[sub]])
        accx = _cap_accept(xkeys, xwgt, xload, xcap)
        okj = accx[:L] & (accx[L:] | (xwgt[L:] == 0))
        okx = np.zeros(todo.size, bool)
        okx[sub] = okj
        fin = todo[okx]
        if fin.size:
            fb = bin_of[fin].astype(np.int64)
            fw = prop[okx]
            fk = kind[fin]
            w_of[fin] = fw.astype(np.int8)
            np.add.at(xload, cellA[okx], wgtA[okx].astype(np.int64))
            bsel = okx & (wgtB > 0)
            np.add.at(xload, cellB[bsel], 1)
            np.add.at(swcnt, (fb * NW + fw)[fk == 0], 1)
            np.add.at(pwcnt, (fb * NW + fw)[fk == 1], 1)
        todo = todo[~okx]
    if todo.size:
        # move stubborn edges to overflow if room, else fail
        b = bin_of[todo]
        ocnt = np.bincount(bin_of[kind == 2], minlength=NBINS)
        order = np.argsort(b, kind="stable")
        sb = b[order]
        newb = np.ones(sb.size, bool)
        newb[1:] = sb[1:] != sb[:-1]
        gstart = np.maximum.accumulate(np.where(newb, np.arange(sb.size), 0))
        cc = np.arange(sb.size) - gstart
        slots = ocnt[sb] + cc
        if (slots >= OVF_CAP).any():
            raise RuntimeError(f"window-assign stragglers overflow: {todo.size}")
        kind[todo[order]] = 2
        ovf_slot[todo[order]] = slots.astype(np.int32)

    # ---------------- concrete J slots
    J_of = np.full(Etot, -1, np.int64)
    sing = np.where(kind == 0)[0]
    grp = bin_of[sing].astype(np.int64) * NW + w_of[sing]
    order = np.argsort(grp, kind="stable")
    sg = grp[order]
    newg = np.ones(sg.size, bool)
    newg[1:] = sg[1:] != sg[:-1]
    gstart = np.maximum.accumulate(np.where(newg, np.arange(sg.size), 0))
    cc = np.arange(sg.size) - gstart
    J_of[sing[order]] = (sg % NW) * VW + cc

    prs = np.where(kind == 1)[0]
    grp = bin_of[prs].astype(np.int64) * NW + w_of[prs]
    order = np.argsort(grp, kind="stable")
    sg = grp[order]
    newg = np.ones(sg.size, bool)
    newg[1:] = sg[1:] != sg[:-1]
    gstart = np.maximum.accumulate(np.where(newg, np.arange(sg.size), 0))
    cc = np.arange(sg.size) - gstart
    pg = (sg % NW)  # pair-group 0 or 1
    base = np.where(pg == 0, P_LO, P_LO + 2 * PEW)
    J_of[prs[order]] = base + cc

    ovfm = kind == 2
    J_of[ovfm] = OVF_LO + ovf_slot[ovfm]
    w_of[ovfm] = (J_of[ovfm] // VW).astype(np.int8)

    return dict(ip0=ip0, ip1=ip1, c0=c0, c1=c1, kind=kind,
                bin=bin_of, w=w_of, J=J_of)


def _prep_inputs(embedding, edges, u, W_emb, b_emb, W_edge, b_edge):
    embedding = np.ascontiguousarray(np.asarray(embedding, dtype=np.float32))
    edges = np.asarray(edges).astype(np.int64)
    u = np.asarray(u, dtype=np.float32)
    W_emb = np.asarray(W_emb, dtype=np.float32)
    b_emb = np.asarray(b_emb, dtype=np.float32)
    W_edge = np.asarray(W_edge, dtype=np.float32)
    b_edge = np.asarray(b_edge, dtype=np.float32)

    wsym = 0.5 * (W_edge[:HID, 0] + W_edge[HID:, 0])
    bias2 = b_emb.reshape(2, 128).T.copy()
    wsym2 = wsym.reshape(2, 128).T.copy()
    bedge = np.tile(np.array([[b_edge[0], 1.0 - BIAS, BIAS]], np.float32), (128, 1))
    iota16 = (np.arange(128) % 16).astype(np.float32)[:, None]
    e8h = (np.arange(128)[:, None] // 16 == np.arange(8)[None, :]).astype(np.float16)
    e8f = e8h.astype(np.float32)

    rng = np.random.default_rng(12345)
    A = _assign_edges(edges[0], edges[1], rng)
    kind, bin_of, w_of, J_of = A["kind"], A["bin"], A["w"], A["J"]
    ip0, ip1, c0, c1 = A["ip0"], A["ip1"], A["c0"], A["c1"]
    o0 = (ip0 % CHUNK).astype(np.int64)
    o1 = (ip1 % CHUNK).astype(np.int64)
    core_of = (bin_of // 8).astype(np.int64)
    g_of = (bin_of % 8).astype(np.int64)

    # ----- per-core index maps
    idx1 = np.full((NCORES, 128, CHUNK), -1, np.int16)
    idx2 = np.full((NCORES, 128, XW), -1, np.int16)

    # refs of the scatter path: singles contribute (row c0) and (row c1);
    # pairs contribute (row c0, col J) and (row c0, col J+1).
    m = kind != 2
    sing = m & (kind == 0)
    prs = m & (kind == 1)
    r_core = np.concatenate([core_of[sing], core_of[sing], core_of[prs], core_of[prs]])
    r_g = np.concatenate([g_of[sing], g_of[sing], g_of[prs], g_of[prs]])
    r_cls = np.concatenate([c0[sing], c1[sing], c0[prs], c0[prs]])
    r_off = np.concatenate([o0[sing], o1[sing], o0[prs], o1[prs]])
    r_J = np.concatenate([J_of[sing], J_of[sing], J_of[prs], J_of[prs] + PEW])
    r_w = (r_J // VW).astype(np.int64)

    row = (r_core * 8 + r_g) * 16 + r_cls          # global row id [0, 1024)
    cell = row * NW + r_w
    order = np.argsort(cell, kind="stable")
    sc = cell[order]
    newc = np.ones(sc.size, bool)
    newc[1:] = sc[1:] != sc[:-1]
    gstart = np.maximum.accumulate(np.where(newc, np.arange(sc.size), 0))
    cc = np.arange(sc.size) - gstart
    assert cc.max() < XS, f"X section overflow: {cc.max()}"
    xpos = r_w[order] * XS + cc                    # position within X row
    p_part = (r_g[order] * 16 + r_cls[order])      # partition 0..127
    idx1[r_core[order], p_part, r_off[order]] = xpos.astype(np.int16)
    jloc = (r_J[order] - r_w[order] * VW)
    idx2[r_core[order], p_part, xpos] = jloc.astype(np.int16)

    # ----- overflow lane maps
    ogidxa = np.zeros((NCORES, 128, OVF_CAP), np.int16)
    ogidxb = np.zeros((NCORES, 128, OVF_CAP), np.int16)
    ocfa = np.full((NCORES, 128, PEW), 255, np.uint8)
    ocfb = np.full((NCORES, 128, PEW), 255, np.uint8)
    ov = np.where(kind == 2)[0]
    if ov.size:
        oc = core_of[ov]
        og = g_of[ov]
        slot = (J_of[ov] - OVF_LO).astype(np.int64)
        assert (slot >= 0).all() and (slot < OVF_CAP).all()
        # wrapped gather idx layout: col i -> partition 16g + i%16, pos i//16
        ogidxa[oc, og * 16 + slot % 16, slot // 16] = o0[ov].astype(np.int16)
        ogidxb[oc, og * 16 + slot % 16, slot // 16] = o1[ov].astype(np.int16)
        for q in range(16):
            ocfa[oc, og * 16 + q, slot] = c0[ov].astype(np.uint8)
            ocfb[oc, og * 16 + q, slot] = c1[ov].astype(np.uint8)
    # wrapped layout already written into the first OVF_CAP//16 columns
    ogidxa = ogidxa[:, :, :OVF_CAP // 16]
    ogidxb = ogidxb[:, :, :OVF_CAP // 16]

    # ----- u in folded layout + output mapping
    part_of = g_of * 16 + J_of // FOLD
    col_of = J_of % FOLD
    uu = np.full((NCORES, 128, FOLD), 0.5, np.float32)
    uu[core_of, part_of, col_of] = u
    outmap = (core_of * 128 + part_of) * FOLD + col_of   # into concat outputs

    in_maps = []
    for c in range(NCORES):
        base = np.zeros((IN_DIM, NLOC_PAD), np.float32)
        base[:, :NLOC] = embedding[NLOC * c:NLOC * (c + 1)].T
        embT = np.ascontiguousarray(
            base.reshape(4, 128, SLABS, 512).transpose(2, 0, 1, 3).reshape(4 * SLABS * 128, 512))
        in_maps.append({
            "embT": embT,
            "Wt": W_emb,
            "bias2": bias2,
            "wsym2": wsym2,
            "bedge": bedge,
            "iota16": iota16,
            "e8h": e8h,
            "e8f": e8f,
            "idx1": np.ascontiguousarray(idx1[c]),
            "idx2": np.ascontiguousarray(idx2[c]),
            "ogidxa": np.ascontiguousarray(ogidxa[c]),
            "ogidxb": np.ascontiguousarray(ogidxb[c]),
            "ocfa": np.ascontiguousarray(ocfa[c]),
            "ocfb": np.ascontiguousarray(ocfb[c]),
            "uu": np.ascontiguousarray(uu[c]),
        })
    return in_maps, outmap


def kernel(embedding, edges, u, W_emb, b_emb, W_edge, b_edge, _trace=False):
    if "prog" not in _PROGRAM_CACHE:
        _PROGRAM_CACHE["prog"] = _build_program()
    nc = _PROGRAM_CACHE["prog"]
    in_maps, outmap = _prep_inputs(embedding, edges, u, W_emb, b_emb, W_edge, b_edge)
    res = run_bass_kernel_spmd(nc, in_maps, core_ids=list(range(NCORES)), trace=_trace)
    allout = np.concatenate([res.results[c]["out"].reshape(-1) for c in range(NCORES)])
    full = allout[outmap].astype(np.float32)
    if _trace:
        kernel._last_results = res
    return full


# revision 21
# speedup vs baseline: 1.1087x; 1.1087x over previous
"""Trainium2 Bass kernel for nn_Discriminator (GNN message passing).

Math (reference):
    h   = relu(embedding @ W_emb + b_emb)          # [N, HID]
    w_sym = 0.5*(W_edge[:HID,0] + W_edge[HID:,0])  # [HID]
    raw = (h[e0] + h[e1]) @ w_sym + b_edge         # [E]
    out = sigmoid(logit(eps) + raw),  eps = (2B-1)*u + (1-B)

Algebraic reduction: raw[e] = s[e0] + s[e1] + b_edge with per-node scalar
s = h @ w_sym, so the edge stage only needs per-node scalars from a 53k
table.

Distribution (8 NeuronCores):
  - node GEMM sharded over N (6250 nodes/core, padded 6656)
  - AllGather of s -> full table on every core
  - edges assigned to cores/slots by the host (output unpermuted on host)

Edge stage (the fast part, replacing the v1 ap_gather approach):
  The s-table sits in SBUF as [128, 3328] fp16: partition 16g+q holds
  table chunk q (replicated over the 8 gpsimd groups g).  Each edge is
  assigned a slot (group g, column J); its two endpoint values are routed
  into V[16g+c, J] (c = endpoint's chunk) by two local_scatter hops
  (table -> X -> V), both with host-precomputed index maps.  A single
  ones-vector matmul per 450-column window sums the 16 chunk rows of each
  group into PSUM, yielding raw[g, J].  local_scatter runs vectorized in
  GPSIMD local RAM (~0.3ns/elem streamed) instead of ap_gather's ~28ns
  per random SBUF read, which was the v1 bottleneck (~700us).
  Edges whose endpoints collide in the same table chunk use even/odd
  column pairs (summed by a strided DVE pass); assignment stragglers and
  self-loops go through a tiny ap_gather overflow lane (64 slots/group).
"""

import os
import sys
import types
import contextlib
import ctypes

sys.path.insert(0, "/opt/trn_rl_repo")

import numpy as np

import concourse.bass as bass
import concourse.mybir as mybir
import concourse.tile as tile
import concourse.bacc as bacc
from concourse.bass_utils import run_bass_kernel_spmd

# ---------------------------------------------------------------- constants
N, IN_DIM, HID, E = 50000, 512, 256, 800000
NCORES = 8
BIAS = 0.0001

NLOC = N // NCORES          # 6250 real nodes per core
SLABS = 13                  # s staging rows of 512 (13*512 = 6656)
NLOC_PAD = SLABS * 512      # 6656 padded local nodes
RANK_PAD = SLABS * 512
NPAD = RANK_PAD * NCORES    # 53248 = 16*3328
CHUNK = NPAD // 16          # 3328 table entries per chunk

J = 14400                   # slot columns per group
PEW = 450                   # PE window
NPE = J // PEW              # 32 PE windows
VW = 1800                   # V piece width (scatter2 dst)
NW = J // VW                # 8 V pieces
XS = 254                    # X section per window
XW = NW * XS                # 2032  (2032*32 = 65024 < 65536)
S_CAP = 12150               # singles region J in [0, 12150)
P_LO, P_HI = 12150, 13950   # pairs region (even/odd J pairs)
OVF_LO = 13950              # overflow region start
OVF_CAP = 64                # overflow slots per group (gather 64 idx x2)
FOLD = J * 8 // 128         # 900 folded columns per partition

f32 = mybir.dt.float32
f16 = mybir.dt.float16
f32r = mybir.dt.float32r
i16 = mybir.dt.int16
u8 = mybir.dt.uint8


def _install_ntff_hook():
    """Provide antenv.axon_hooks (absent in this image) so trace=True works."""
    if "antenv.axon_hooks" in sys.modules:
        return
    try:
        lib = ctypes.CDLL("/opt/axon/libaxon_pjrt.so")
    except OSError:
        return
    if not hasattr(lib, "axon_start_nrt_profile"):
        return
    lib.axon_start_nrt_profile.argtypes = [ctypes.POINTER(ctypes.c_int64), ctypes.c_size_t]
    lib.axon_start_nrt_profile.restype = ctypes.c_int64
    lib.axon_stop_nrt_profile.argtypes = [ctypes.c_char_p]
    lib.axon_stop_nrt_profile.restype = ctypes.c_int64

    @contextlib.contextmanager
    def _hook(output_dir, device_ids):
        import jax
        jax.devices()
        if device_ids:
            ids = (ctypes.c_int64 * len(device_ids))(*device_ids)
            rc = lib.axon_start_nrt_profile(ids, len(device_ids))
        else:
            rc = lib.axon_start_nrt_profile(None, 0)
        if rc != 0:
            raise RuntimeError(f"axon_start_nrt_profile rc={rc}")
        try:
            yield
        finally:
            n = lib.axon_stop_nrt_profile(str(output_dir).encode())
            print(f"profile: {n} file(s) written to {output_dir}", file=sys.stderr)

    mod = types.ModuleType("antenv.axon_hooks")
    mod.get_axon_ntff_profile_hook = lambda: _hook
    mod.set_axon_ntff_profile_hook = lambda h: None
    sys.modules["antenv.axon_hooks"] = mod


_install_ntff_hook()

_PROGRAM_CACHE = {}


def _build_program():
    nc = bacc.Bacc(None)

    embT = nc.dram_tensor("embT", [4 * SLABS * 128, 512], f32r, kind="ExternalInput")
    Wt = nc.dram_tensor("Wt", [IN_DIM, HID], f32r, kind="ExternalInput")
    bias2 = nc.dram_tensor("bias2", [128, 2], f32, kind="ExternalInput")
    wsym2 = nc.dram_tensor("wsym2", [128, 2], f32r, kind="ExternalInput")
    bedge = nc.dram_tensor("bedge", [128, 3], f32, kind="ExternalInput")
    iota16 = nc.dram_tensor("iota16", [128, 1], f32, kind="ExternalInput")
    e8h = nc.dram_tensor("e8h", [128, 8], f16, kind="ExternalInput")
    e8f = nc.dram_tensor("e8f", [128, 8], f32r, kind="ExternalInput")
    idx1 = nc.dram_tensor("idx1", [128, CHUNK], i16, kind="ExternalInput")
    idx2 = nc.dram_tensor("idx2", [128, XW], i16, kind="ExternalInput")
    ogidxa = nc.dram_tensor("ogidxa", [128, OVF_CAP // 16], i16, kind="ExternalInput")
    ogidxb = nc.dram_tensor("ogidxb", [128, OVF_CAP // 16], i16, kind="ExternalInput")
    ocfa = nc.dram_tensor("ocfa", [128, PEW], u8, kind="ExternalInput")
    ocfb = nc.dram_tensor("ocfb", [128, PEW], u8, kind="ExternalInput")
    uu = nc.dram_tensor("uu", [128, FOLD], f32, kind="ExternalInput")
    out = nc.dram_tensor("out", [128, FOLD], f32, kind="ExternalOutput")

    with tile.TileContext(nc) as tc:
        with (
            tc.tile_pool(name="const", bufs=1) as constp,
            tc.tile_pool(name="w", bufs=1) as wp,
            tc.tile_pool(name="emb", bufs=3) as embp,
            tc.tile_pool(name="h", bufs=2) as hp,
            tc.tile_pool(name="s", bufs=3) as sp,
            tc.tile_pool(name="tab", bufs=1) as tabp,
            tc.tile_pool(name="x", bufs=1) as xp,
            tc.tile_pool(name="v", bufs=3) as vp,
            tc.tile_pool(name="fin", bufs=1) as finp,
            tc.tile_pool(name="psA", bufs=2, space="PSUM") as psA,
            tc.tile_pool(name="psS", bufs=2, space="PSUM") as psS,
            tc.tile_pool(name="psR", bufs=4, space="PSUM") as psR,
            tc.tile_pool(name="dram", bufs=1, space="DRAM") as dramp,
        ):
            # ---------------- constants into SBUF
            t_bias2 = constp.tile([128, 2], f32)
            nc.sync.dma_start(t_bias2[:], bias2[:])
            t_wsym2 = constp.tile([128, 2], f32r)
            nc.sync.dma_start(t_wsym2[:], wsym2[:])
            t_bedge = constp.tile([128, 3], f32)
            nc.sync.dma_start(t_bedge[:], bedge[:])
            t_iota16 = constp.tile([128, 1], f32)
            nc.sync.dma_start(t_iota16[:], iota16[:])
            t_e8h = constp.tile([128, 8], f16)
            nc.sync.dma_start(t_e8h[:], e8h[:])
            t_e8f = constp.tile([128, 8], f32r)
            nc.sync.dma_start(t_e8f[:], e8f[:])
            t_W = wp.tile([128, 4 * HID], f32r)
            for k in range(4):
                nc.sync.dma_start(t_W[:, k * HID:(k + 1) * HID], Wt[128 * k:128 * (k + 1), :])
            # index maps on the Activation DMA queue to spread dispatch
            t_idx1 = constp.tile([128, CHUNK], i16)
            nc.scalar.dma_start(t_idx1[:], idx1[:])
            t_idx2 = constp.tile([128, XW], i16)
            nc.scalar.dma_start(t_idx2[:], idx2[:])
            t_ogidxa = constp.tile([128, OVF_CAP // 16], i16)
            nc.scalar.dma_start(t_ogidxa[:], ogidxa[:])
            t_ogidxb = constp.tile([128, OVF_CAP // 16], i16)
            nc.scalar.dma_start(t_ogidxb[:], ogidxb[:])
            t_ocfa = constp.tile([128, PEW], u8)
            nc.scalar.dma_start(t_ocfa[:], ocfa[:])
            t_ocfb = constp.tile([128, PEW], u8)
            nc.scalar.dma_start(t_ocfb[:], ocfb[:])
            t_u = finp.tile([128, FOLD], f32)
            nc.scalar.dma_start(t_u[:], uu[:])

            # ---------------- stage A: s = relu(emb @ W + b) @ w_sym
            d_sin = dramp.tile([SLABS, 512], f16)
            for si in range(SLABS):
                t_embs = embp.tile([128, 4 * 512], f32r, tag="embs")
                for k in range(4):
                    blk = (si * 4 + k) * 128
                    nc.sync.dma_start(
                        t_embs[:, k * 512:(k + 1) * 512],
                        embT[blk:blk + 128, :],
                    )
                ps_s = psS.tile([1, 512], f32, tag="ps_s")
                for H in range(2):
                    ps_h = psA.tile([128, 512], f32, tag="ps_h")
                    for k in range(4):
                        nc.tensor.matmul(
                            ps_h[:],
                            lhsT=t_W[:, k * HID + 128 * H:k * HID + 128 * (H + 1)],
                            rhs=t_embs[:, k * 512:(k + 1) * 512],
                            start=(k == 0),
                            stop=(k == 3),
                        )
                    t_h = hp.tile([128, 512], f32r, tag="h")
                    nc.scalar.activation(
                        t_h[:], ps_h[:],
                        mybir.ActivationFunctionType.Relu,
                        bias=t_bias2[:, H:H + 1],
                    )
                    nc.tensor.matmul(
                        ps_s[:1, :],
                        lhsT=t_wsym2[:, H:H + 1],
                        rhs=t_h[:],
                        start=(H == 0),
                        stop=(H == 1),
                    )
                t_sst = sp.tile([1, 512], f16, tag="sst")
                nc.vector.tensor_copy(t_sst[:1, :], ps_s[:1, :])
                nc.sync.dma_start(d_sin[si:si + 1, :], t_sst[:1, :])

            # ---------------- stage B: AllGather s (fp16) -> full table
            d_sout = dramp.tile([16, CHUNK], f16)
            nc.gpsimd.collective_compute(
                "AllGather",
                mybir.AluOpType.bypass,
                ins=[d_sin[:].opt()],
                outs=[d_sout[:].opt()],
                replica_groups=[list(range(NCORES))],
            )
            t_tabh = tabp.tile([128, CHUNK], f16)
            for g in range(8):
                nc.sync.dma_start(t_tabh[16 * g:16 * (g + 1), :], d_sout[:, :])
            # f32 table (overflow gather source) cast up from fp16
            t_tabf = tabp.tile([128, CHUNK], f32)
            nc.vector.tensor_copy(t_tabf[:], t_tabh[:])

            # ---------------- overflow lane: tiny ap_gather (async on gpsimd)
            t_oga = tabp.tile([128, PEW], f32)
            t_ogb = tabp.tile([128, PEW], f32)
            nc.vector.memset(t_oga[:, OVF_CAP:], 0.0)
            nc.vector.memset(t_ogb[:, OVF_CAP:], 0.0)
            tabf3 = t_tabf[:].rearrange("p (n d) -> p n d", d=1)
            nc.gpsimd.ap_gather(
                t_oga[:, :OVF_CAP].rearrange("p (n d) -> p n d", d=1),
                tabf3, t_ogidxa[:],
                channels=128, num_elems=CHUNK, d=1, num_idxs=OVF_CAP)
            nc.gpsimd.ap_gather(
                t_ogb[:, :OVF_CAP].rearrange("p (n d) -> p n d", d=1),
                tabf3, t_ogidxb[:],
                channels=128, num_elems=CHUNK, d=1, num_idxs=OVF_CAP)
            # masks: keep only the partition whose chunk matches
            t_cfa = tabp.tile([128, PEW], f32)
            nc.vector.tensor_copy(t_cfa[:], t_ocfa[:])
            t_cfb = tabp.tile([128, PEW], f32)
            nc.vector.tensor_copy(t_cfb[:], t_ocfb[:])
            t_mA = tabp.tile([128, PEW], f32r)
            nc.vector.scalar_tensor_tensor(
                t_mA[:], in0=t_cfa[:], scalar=t_iota16[:, 0:1], in1=t_oga[:],
                op0=mybir.AluOpType.is_equal, op1=mybir.AluOpType.mult)
            t_mB = tabp.tile([128, PEW], f32r)
            nc.vector.scalar_tensor_tensor(
                t_mB[:], in0=t_cfb[:], scalar=t_iota16[:, 0:1], in1=t_ogb[:],
                op0=mybir.AluOpType.is_equal, op1=mybir.AluOpType.mult)

            # ---------------- edge main: scatter1 (table -> X)
            t_x = xp.tile([128, XW], f16)
            nc.gpsimd.local_scatter(
                t_x[:], t_tabh[:], t_idx1[:],
                channels=128, num_elems=XW, num_idxs=CHUNK)

            # ---------------- scatter2 pieces + PE reduce
            # window win [8, 450] lands in folded t_raw at partitions
            # {16g + win//2}, cols [450*(win%2), +450)
            t_raw = finp.tile([128, FOLD], f32)
            raw3 = t_raw[:].rearrange("(g c) f -> g c f", g=8)

            def fold_dst(win):
                return raw3[:, win // 2, PEW * (win % 2):PEW * (win % 2 + 1)]

            for w in range(NW):
                t_v = vp.tile([128, VW], f16, tag="v")
                nc.gpsimd.local_scatter(
                    t_v[:], t_x[:, XS * w:XS * (w + 1)], t_idx2[:, XS * w:XS * (w + 1)],
                    channels=128, num_elems=VW, num_idxs=XS)
                for k in range(4):
                    win = 4 * w + k
                    ps_r = psR.tile([8, PEW], f32, tag="ps_r")
                    last_win = (win == NPE - 1)
                    nc.tensor.matmul(
                        ps_r[:], lhsT=t_e8h[:],
                        rhs=t_v[:, PEW * k:PEW * (k + 1)],
                        start=True, stop=not last_win)
                    if last_win:  # overflow strips accumulate into win 31
                        nc.tensor.matmul(ps_r[:], lhsT=t_e8f[:], rhs=t_mA[:],
                                         start=False, stop=False)
                        nc.tensor.matmul(ps_r[:], lhsT=t_e8f[:], rhs=t_mB[:],
                                         start=False, stop=True)
                    t_r = sp.tile([8, PEW], f32, tag="raw")
                    nc.vector.tensor_copy(t_r[:], ps_r[:])
                    if win in (27, 29):
                        # pair window A: hold; summed with partner window B
                        pend_pair = t_r
                        continue
                    if win in (28, 30):
                        # pairs: raw[J] = raw_A[J] + raw_B[J+450]
                        t_f = sp.tile([8, PEW], f32, tag="fix")
                        nc.vector.tensor_add(t_f[:], pend_pair[:], t_r[:])
                        nc.sync.dma_start(fold_dst(win - 1), t_f[:])
                    nc.sync.dma_start(fold_dst(win), t_r[:])

            # ---------------- gate: logit(eps) + raw, sigmoid
            a = 1.0 - 2.0 * BIAS
            t_l1 = finp.tile([128, FOLD], f32)
            nc.scalar.activation(t_l1[:], t_u[:], mybir.ActivationFunctionType.Ln,
                                 bias=t_bedge[:, 1:2], scale=-a)
            t_l2 = finp.tile([128, FOLD], f32)
            nc.scalar.activation(t_l2[:], t_u[:], mybir.ActivationFunctionType.Ln,
                                 bias=t_bedge[:, 2:3], scale=a)
            t_gate = finp.tile([128, FOLD], f32)
            nc.vector.tensor_sub(t_gate[:], t_l1[:], t_l2[:])
            t_gate2 = finp.tile([128, FOLD], f32)
            nc.vector.tensor_add(t_gate2[:], t_gate[:], t_raw[:])
            t_out = finp.tile([128, FOLD], f32)
            nc.scalar.activation(t_out[:], t_gate2[:], mybir.ActivationFunctionType.Sigmoid,
                                 bias=t_bedge[:, 0:1])
            nc.sync.dma_start(out[:, :], t_out[:])

    nc.finalize()
    return nc


# ================================================================ host prep
def _assign_edges(e0, e1, rng):
    """Assign each edge to (bin=core*8+group, kind, J-slot, window).

    Returns dict of per-edge arrays: bin, kind (0=single,1=pair,2=ovf),
    w (V piece), J (slot column; for pairs the even column).
    Constraints honored:
      - per (bin, node): at most one reference (scatter1 is one cell per
        (row, node));
      - per (bin, window, kind): slot-region capacities;
      - per (bin, class, window): at most XS refs (X section capacity);
      - per bin: at most OVF_CAP overflow edges.
    """
    Etot = e0.shape[0]
    ip0 = (RANK_PAD * (e0 // NLOC) + (e0 % NLOC)).astype(np.int64)
    ip1 = (RANK_PAD * (e1 // NLOC) + (e1 % NLOC)).astype(np.int64)
    c0 = (ip0 // CHUNK).astype(np.int32)
    c1 = (ip1 // CHUNK).astype(np.int32)
    selfloop = e0 == e1
    pair = (c0 == c1) & ~selfloop
    kind = np.where(pair, 1, 0).astype(np.int8)
    kind[selfloop] = 2

    NBINS = 64
    bin_of = np.full(Etot, -1, np.int32)
    used = np.zeros(NBINS * NPAD, bool)
    sing_cnt = np.zeros(NBINS, np.int64)
    pair_cnt = np.zeros(NBINS, np.int64)
    S_BIN_CAP, P_BIN_CAP = S_CAP, (P_HI - P_LO) // 2

    un = np.where(~selfloop)[0]
    for _ in range(60):
        if un.size == 0:
            break
        prop = rng.integers(0, NBINS, un.size).astype(np.int64)
        k0 = prop * NPAD + ip0[un]
        k1 = prop * NPAD + ip1[un]
        ok = (~used[k0]) & (~used[k1])
        cand = np.where(ok)[0]
        if cand.size == 0:
            continue
        ck0, ck1 = k0[cand], k1[cand]
        L = cand.size
        flat = np.concatenate([ck0, ck1])
        srt = np.argsort(flat, kind="stable")
        fs = flat[srt]
        firstpos = np.ones(2 * L, bool)
        firstpos[1:] = fs[1:] != fs[:-1]
        first = np.zeros(2 * L, bool)
        first[srt] = firstpos
        win = first[:L] & first[L:]
        wc = cand[win]
        if wc.size == 0:
            continue
        # capacity per (bin, kind)
        wbin = prop[wc]
        wkind = kind[un[wc]]
        order = np.lexsort((np.arange(wc.size), wkind, wbin))
        sb, sk = wbin[order], wkind[order]
        grp = sb * 2 + sk
        newgrp = np.ones(sb.size, bool)
        newgrp[1:] = grp[1:] != grp[:-1]
        gstart = np.maximum.accumulate(np.where(newgrp, np.arange(sb.size), 0))
        cc = np.arange(sb.size) - gstart
        already = np.where(sk == 0, sing_cnt[sb], pair_cnt[sb])
        cap = np.where(sk == 0, S_BIN_CAP, P_BIN_CAP)
        acc_sorted = cc + already < cap
        acc = np.zeros(wc.size, bool)
        acc[order] = acc_sorted
        fin = wc[acc]
        if fin.size == 0:
            continue
        eidx = un[fin]
        b = prop[fin]
        bin_of[eidx] = b
        used[b * NPAD + ip0[eidx]] = True
        used[b * NPAD + ip1[eidx]] = True
        np.add.at(sing_cnt, b[kind[eidx] == 0], 1)
        np.add.at(pair_cnt, b[kind[eidx] == 1], 1)
        un = un[~np.isin(un, eidx, assume_unique=True)]

    # leftovers + selfloops -> overflow
    ovf = np.concatenate([un, np.where(selfloop)[0]])
    kind[ovf] = 2
    if ovf.size > 0:
        prop = rng.permutation(ovf.size) % NBINS
        order = np.argsort(prop, kind="stable")
        sb = prop[order]
        newb = np.ones(sb.size, bool)
        newb[1:] = sb[1:] != sb[:-1]
        gstart = np.maximum.accumulate(np.where(newb, np.arange(sb.size), 0))
        cc = np.arange(sb.size) - gstart
        if (cc >= OVF_CAP).any():
            raise RuntimeError(f"overflow capacity exceeded: {ovf.size} stragglers")
        bin_of[ovf[order]] = sb
        ovf_slot = np.zeros(Etot, np.int32)
        ovf_slot[ovf[order]] = cc.astype(np.int32)
    else:
        ovf_slot = np.zeros(Etot, np.int32)

    # ---------------- window assignment (for singles and pairs)
    # pairs live in PE windows 27..30 as (J, J+450) partners:
    #   pair-group 0: J_A in [12150,12600) [piece 6], J_B=J_A+450 [piece 7]
    #   pair-group 1: J_A in [13050,13500) [piece 7], J_B=J_A+450 [piece 7]
    w_of = np.full(Etot, -1, np.int8)
    # slot capacities per window (singles); pairs capacity per pair-group
    s_wcap = np.array([1800] * 6 + [1350, 0], np.int64)
    p_wcap = np.array([450, 450] + [0] * (NW - 2), np.int64)
    xload = np.zeros(NBINS * 16 * NW, np.int64)   # refs per (bin, class, window)
    swcnt = np.zeros(NBINS * NW, np.int64)
    pwcnt = np.zeros(NBINS * NW, np.int64)
    def _cap_accept(keys, weights, counts, caps):
        """Accept a prefix per key group such that counts[key]+cumw <= caps."""
        order = np.argsort(keys, kind="stable")
        sk = keys[order]
        csw = np.cumsum(weights[order])
        newg = np.ones(sk.size, bool)
        newg[1:] = sk[1:] != sk[:-1]
        gidx = np.where(newg, np.arange(sk.size), 0)
        gstart = np.maximum.accumulate(gidx)
        base = np.where(gstart > 0, csw[np.maximum(gstart - 1, 0)], 0.0)
        base[gstart == 0] = 0.0
        cumw = csw - base  # inclusive cumulative weight within group
        ok_sorted = counts[sk] + cumw <= caps[sk]
        ok = np.zeros(keys.size, bool)
        ok[order] = ok_sorted
        return ok

    todo = np.where(kind != 2)[0]
    for _ in range(60):
        if todo.size == 0:
            break
        k = kind[todo]
        pw = np.where(k == 0, 1.0, 0.0)[:, None] * s_wcap[None, :] + \
             np.where(k == 1, 1.0, 0.0)[:, None] * p_wcap[None, :]
        r = rng.random(todo.size)
        cdf = np.cumsum(pw, axis=1)
        cdf /= cdf[:, -1:]
        prop = (r[:, None] < cdf).argmax(axis=1).astype(np.int64)
        b = bin_of[todo].astype(np.int64)
        bw = b * NW + prop
        # slot capacity per (bin, w, kind)
        wcnt_all = np.concatenate([swcnt, pwcnt])  # (bin*NW+w) + 512*kind
        keys_s = bw + (NBINS * NW) * k.astype(np.int64)
        cap_lookup = np.empty(2 * NBINS * NW, np.int64)
        cap_lookup[:NBINS * NW] = np.tile(s_wcap, NBINS)
        cap_lookup[NBINS * NW:] = np.tile(p_wcap, NBINS)
        ok1 = _cap_accept(keys_s, np.ones(todo.size), wcnt_all, cap_lookup)
        # X-load: joint check on both ref cells (phantom consumption on
        # partial accept is conservative and simply retries next round).
        # singles: (c0, w) and (c1, w); pair group 0: (c0, 6) and (c0, 7);
        # pair group 1: (c0, 7) weight 2.
        rowb0 = b * 16 + c0[todo]
        rowb1 = b * 16 + c1[todo]
        cellA = np.where(k == 0, rowb0 * NW + prop,
                         np.where(prop == 0, rowb0 * NW + 6, rowb0 * NW + 7))
        wgtA = np.where((k == 1) & (prop == 1), 2.0, 1.0)
        cellB = np.where(k == 0, rowb1 * NW + prop, rowb0 * NW + 7)
        wgtB = np.where(k == 0, 1.0, np.where(prop == 0, 1.0, 0.0))
        xcap = np.full(NBINS * 16 * NW, XS, np.int64)
        sub = np.where(ok1)[0]
        L = sub.size
        xkeys = np.concatenate([cellA[sub], cellB[sub]])
        xwgt = np.concatenate([wgtA[sub], wgtB[sub]])
        accx = _cap_accept(xkeys, xwgt, xload, xcap)
        okj = accx[:L] & (accx[L:] | (xwgt[L:] == 0))
        okx = np.zeros(todo.size, bool)
        okx[sub] = okj
        fin = todo[okx]
        if fin.size:
            fb = bin_of[fin].astype(np.int64)
            fw = prop[okx]
            fk = kind[fin]
            w_of[fin] = fw.astype(np.int8)
            np.add.at(xload, cellA[okx], wgtA[okx].astype(np.int64))
            bsel = okx & (wgtB > 0)
            np.add.at(xload, cellB[bsel], 1)
            np.add.at(swcnt, (fb * NW + fw)[fk == 0], 1)
            np.add.at(pwcnt, (fb * NW + fw)[fk == 1], 1)
        todo = todo[~okx]
    if todo.size:
        # move stubborn edges to overflow if room, else fail
        b = bin_of[todo]
        ocnt = np.bincount(bin_of[kind == 2], minlength=NBINS)
        order = np.argsort(b, kind="stable")
        sb = b[order]
        newb = np.ones(sb.size, bool)
        newb[1:] = sb[1:] != sb[:-1]
        gstart = np.maximum.accumulate(np.where(newb, np.arange(sb.size), 0))
        cc = np.arange(sb.size) - gstart
        slots = ocnt[sb] + cc
        if (slots >= OVF_CAP).any():
            raise RuntimeError(f"window-assign stragglers overflow: {todo.size}")
        kind[todo[order]] = 2
        ovf_slot[todo[order]] = slots.astype(np.int32)

    # ---------------- concrete J slots
    J_of = np.full(Etot, -1, np.int64)
    sing = np.where(kind == 0)[0]
    grp = bin_of[sing].astype(np.int64) * NW + w_of[sing]
    order = np.argsort(grp, kind="stable")
    sg = grp[order]
    newg = np.ones(sg.size, bool)
    newg[1:] = sg[1:] != sg[:-1]
    gstart = np.maximum.accumulate(np.where(newg, np.arange(sg.size), 0))
    cc = np.arange(sg.size) - gstart
    J_of[sing[order]] = (sg % NW) * VW + cc

    prs = np.where(kind == 1)[0]
    grp = bin_of[prs].astype(np.int64) * NW + w_of[prs]
    order = np.argsort(grp, kind="stable")
    sg = grp[order]
    newg = np.ones(sg.size, bool)
    newg[1:] = sg[1:] != sg[:-1]
    gstart = np.maximum.accumulate(np.where(newg, np.arange(sg.size), 0))
    cc = np.arange(sg.size) - gstart
    pg = (sg % NW)  # pair-group 0 or 1
    base = np.where(pg == 0, P_LO, P_LO + 2 * PEW)
    J_of[prs[order]] = base + cc

    ovfm = kind == 2
    J_of[ovfm] = OVF_LO + ovf_slot[ovfm]
    w_of[ovfm] = (J_of[ovfm] // VW).astype(np.int8)

    return dict(ip0=ip0, ip1=ip1, c0=c0, c1=c1, kind=kind,
                bin=bin_of, w=w_of, J=J_of)


def _prep_inputs(embedding, edges, u, W_emb, b_emb, W_edge, b_edge):
    embedding = np.ascontiguousarray(np.asarray(embedding, dtype=np.float32))
    edges = np.asarray(edges).astype(np.int64)
    u = np.asarray(u, dtype=np.float32)
    W_emb = np.asarray(W_emb, dtype=np.float32)
    b_emb = np.asarray(b_emb, dtype=np.float32)
    W_edge = np.asarray(W_edge, dtype=np.float32)
    b_edge = np.asarray(b_edge, dtype=np.float32)

    wsym = 0.5 * (W_edge[:HID, 0] + W_edge[HID:, 0])
    bias2 = b_emb.reshape(2, 128).T.copy()
    wsym2 = wsym.reshape(2, 128).T.copy()
    bedge = np.tile(np.array([[b_edge[0], 1.0 - BIAS, BIAS]], np.float32), (128, 1))
    iota16 = (np.arange(128) % 16).astype(np.float32)[:, None]
    e8h = (np.arange(128)[:, None] // 16 == np.arange(8)[None, :]).astype(np.float16)
    e8f = e8h.astype(np.float32)

    rng = np.random.default_rng(12345)
    A = _assign_edges(edges[0], edges[1], rng)
    kind, bin_of, w_of, J_of = A["kind"], A["bin"], A["w"], A["J"]
    ip0, ip1, c0, c1 = A["ip0"], A["ip1"], A["c0"], A["c1"]
    o0 = (ip0 % CHUNK).astype(np.int64)
    o1 = (ip1 % CHUNK).astype(np.int64)
    core_of = (bin_of // 8).astype(np.int64)
    g_of = (bin_of % 8).astype(np.int64)

    # ----- per-core index maps
    idx1 = np.full((NCORES, 128, CHUNK), -1, np.int16)
    idx2 = np.full((NCORES, 128, XW), -1, np.int16)

    # refs of the scatter path: singles contribute (row c0) and (row c1);
    # pairs contribute (row c0, col J) and (row c0, col J+1).
    m = kind != 2
    sing = m & (kind == 0)
    prs = m & (kind == 1)
    r_core = np.concatenate([core_of[sing], core_of[sing], core_of[prs], core_of[prs]])
    r_g = np.concatenate([g_of[sing], g_of[sing], g_of[prs], g_of[prs]])
    r_cls = np.concatenate([c0[sing], c1[sing], c0[prs], c0[prs]])
    r_off = np.concatenate([o0[sing], o1[sing], o0[prs], o1[prs]])
    r_J = np.concatenate([J_of[sing], J_of[sing], J_of[prs], J_of[prs] + PEW])
    r_w = (r_J // VW).astype(np.int64)

    row = (r_core * 8 + r_g) * 16 + r_cls          # global row id [0, 1024)
    cell = row * NW + r_w
    order = np.argsort(cell, kind="stable")
    sc = cell[order]
    newc = np.ones(sc.size, bool)
    newc[1:] = sc[1:] != sc[:-1]
    gstart = np.maximum.accumulate(np.where(newc, np.arange(sc.size), 0))
    cc = np.arange(sc.size) - gstart
    assert cc.max() < XS, f"X section overflow: {cc.max()}"
    xpos = r_w[order] * XS + cc                    # position within X row
    p_part = (r_g[order] * 16 + r_cls[order])      # partition 0..127
    idx1[r_core[order], p_part, r_off[order]] = xpos.astype(np.int16)
    jloc = (r_J[order] - r_w[order] * VW)
    idx2[r_core[order], p_part, xpos] = jloc.astype(np.int16)

    # ----- overflow lane maps
    ogidxa = np.zeros((NCORES, 128, OVF_CAP), np.int16)
    ogidxb = np.zeros((NCORES, 128, OVF_CAP), np.int16)
    ocfa = np.full((NCORES, 128, PEW), 255, np.uint8)
    ocfb = np.full((NCORES, 128, PEW), 255, np.uint8)
    ov = np.where(kind == 2)[0]
    if ov.size:
        oc = core_of[ov]
        og = g_of[ov]
        slot = (J_of[ov] - OVF_LO).astype(np.int64)
        assert (slot >= 0).all() and (slot < OVF_CAP).all()
        # wrapped gather idx layout: col i -> partition 16g + i%16, pos i//16
        ogidxa[oc, og * 16 + slot % 16, slot // 16] = o0[ov].astype(np.int16)
        ogidxb[oc, og * 16 + slot % 16, slot // 16] = o1[ov].astype(np.int16)
        for q in range(16):
            ocfa[oc, og * 16 + q, slot] = c0[ov].astype(np.uint8)
            ocfb[oc, og * 16 + q, slot] = c1[ov].astype(np.uint8)
    # wrapped layout already written into the first OVF_CAP//16 columns
    ogidxa = ogidxa[:, :, :OVF_CAP // 16]
    ogidxb = ogidxb[:, :, :OVF_CAP // 16]

    # ----- u in folded layout + output mapping
    part_of = g_of * 16 + J_of // FOLD
    col_of = J_of % FOLD
    uu = np.full((NCORES, 128, FOLD), 0.5, np.float32)
    uu[core_of, part_of, col_of] = u
    outmap = (core_of * 128 + part_of) * FOLD + col_of   # into concat outputs

    in_maps = []
    for c in range(NCORES):
        base = np.zeros((IN_DIM, NLOC_PAD), np.float32)
        base[:, :NLOC] = embedding[NLOC * c:NLOC * (c + 1)].T
        embT = np.ascontiguousarray(
            base.reshape(4, 128, SLABS, 512).transpose(2, 0, 1, 3).reshape(4 * SLABS * 128, 512))
        in_maps.append({
            "embT": embT,
            "Wt": W_emb,
            "bias2": bias2,
            "wsym2": wsym2,
            "bedge": bedge,
            "iota16": iota16,
            "e8h": e8h,
            "e8f": e8f,
            "idx1": np.ascontiguousarray(idx1[c]),
            "idx2": np.ascontiguousarray(idx2[c]),
            "ogidxa": np.ascontiguousarray(ogidxa[c]),
            "ogidxb": np.ascontiguousarray(ogidxb[c]),
            "ocfa": np.ascontiguousarray(ocfa[c]),
            "ocfb": np.ascontiguousarray(ocfb[c]),
            "uu": np.ascontiguousarray(uu[c]),
        })
    return in_maps, outmap


def kernel(embedding, edges, u, W_emb, b_emb, W_edge, b_edge, _trace=False):
    if "prog" not in _PROGRAM_CACHE:
        _PROGRAM_CACHE["prog"] = _build_program()
    nc = _PROGRAM_CACHE["prog"]
    in_maps, outmap = _prep_inputs(embedding, edges, u, W_emb, b_emb, W_edge, b_edge)
    res = run_bass_kernel_spmd(nc, in_maps, core_ids=list(range(NCORES)), trace=_trace)
    allout = np.concatenate([res.results[c]["out"].reshape(-1) for c in range(NCORES)])
    full = allout[outmap].astype(np.float32)
    if _trace:
        kernel._last_results = res
    return full
